# revision 2
# baseline (speedup 1.0000x reference)
"""4-layer GCN (GroupGCN) on 8 TRN2 NeuronCores.

Strategy:
  - Nodes relabeled by degree rank and striped across the 8 cores (core = rank % 8)
    so every core owns 12500 nodes with an even mix of degrees; edges are assigned
    to the core that owns their dst node.
  - Per layer: each core computes the scaled feature table rows for its own nodes
    (h~ = dinv * (z @ W)) with PE matmuls, all cores AllGather the full table into
    DRAM, then each core aggregates its nodes' in-edges with one indirect-DMA
    gather per supergroup followed by a DVE fold-tree segmented sum (level 1
    bf16->f32, rest f32 for accuracy).
  - Table rows are laid out as row = p*G + g (partition-major) so the per-layer
    shard write from SBUF [P, G*dout] is a single dense DMA.
  - dinv (D^-1/2) factorizes out of the per-edge norm: table rows are pre-scaled
    by dinv[src], the aggregated sum is post-scaled by dinv[dst].
  - x is bulk-loaded into SBUF once (2 DMAs); the edge index table lives in SBUF
    for the whole kernel; epilogue + next-layer table build run per-supergroup,
    pipelined with the remaining gathers/folds of the current layer.
"""

import numpy as np
import ml_dtypes

N_NODES = 100000
N_EDGES = 3200000
IN_DIM = 256
DIMS = [64, 32, 16, 4]
C = 8           # cores
P = 128         # partitions
G = 98          # node groups of 128 per core
PC = G * P      # padded nodes per core (12544)
PG = P * G      # table rows per core (= PC)
NPC = N_NODES // C  # real nodes per core (12500)
MSG_SLOT_BUDGET = 224   # max slots (w*K) per supergroup gather

_COMPILED = {}
LAST_RESULT = None


# ----------------------------------------------------------------------------
# Host-side graph preprocessing
# ----------------------------------------------------------------------------
def _preprocess(edge_index):
    src = edge_index[0].astype(np.int64)
    dst = edge_index[1].astype(np.int64)
    loop = np.arange(N_NODES, dtype=np.int64)
    src = np.concatenate([src, loop])
    dst = np.concatenate([dst, loop])

    deg = np.bincount(dst, minlength=N_NODES).astype(np.float64)
    dinv = np.where(deg > 0, 1.0 / np.sqrt(deg), 0.0).astype(np.float32)

    # degree-rank striping: rank r -> core r % C, slot r // C
    order = np.argsort(deg, kind="stable")          # ascending degree
    core_of_rank = np.arange(N_NODES) % C
    pos_of_rank = np.arange(N_NODES) // C
    new_id = np.empty(N_NODES, dtype=np.int64)
    new_id[order] = core_of_rank * PC + pos_of_rank
    old_of_new = np.full(C * PC, -1, dtype=np.int64)
    old_of_new[new_id] = np.arange(N_NODES)

    src_n = new_id[src]
    dst_n = new_id[dst]

    # sort edges by (dst_new, src_new)
    sort_idx = np.lexsort((src_n, dst_n))
    src_n = src_n[sort_idx]
    dst_n = dst_n[sort_idx]

    # per-node degree in new-id space (length C*PC, dummies have 0)
    deg_n = np.bincount(dst_n, minlength=C * PC)

    # per-(core, group) max degree, then global max across cores -> K_g
    deg_pg = deg_n.reshape(C, G, P)
    K_g = deg_pg.max(axis=(0, 2)).astype(np.int64)  # [G]
    K_g = np.maximum(K_g, 1)

    # pack groups (in order) into supergroups; slots within the IDX array are
    # padded to S0 per supergroup but only the first w*K columns are gathered
    S0 = MSG_SLOT_BUDGET
    sgs = []  # list of (g_start, width, K)
    g0 = 0
    while g0 < G:
        K = int(K_g[g0])
        w = 1
        while (g0 + w < G and
               max(K, int(K_g[g0 + w])) * (w + 1) <= S0 and
               w < 14):
            K = max(K, int(K_g[g0 + w]))
            w += 1
        K = max(K, int(K_g[g0:g0 + w].max()))
        sgs.append((g0, w, K))
        g0 += w

    SK = len(sgs) * S0
    # column offset of each group inside the IDX array
    col_of_group = np.zeros(G, dtype=np.int64)
    for si, (gs, w, K) in enumerate(sgs):
        for i in range(w):
            col_of_group[gs + i] = si * S0 + i * K

    # table row id of a node new_id n: c = n // PC, pos = n % PC,
    # p = pos % 128, g = pos // 128 -> row = c*PG + p*G + g
    def row_of_new(n):
        c = n // PC
        pos = n % PC
        p = pos % P
        g = pos // P
        return c * PG + p * G + g

    ZERO_ROW = int(row_of_new(np.int64(NPC)))  # core 0's first dummy node

    # build IDX [C, P, SK] int32 of table row ids
    IDX = np.full((C, P, SK), ZERO_ROW, dtype=np.int32)
    node_starts = np.zeros(C * PC + 1, dtype=np.int64)
    np.cumsum(deg_n, out=node_starts[1:])
    slot_in_node = np.arange(len(dst_n)) - node_starts[dst_n]
    core_e = dst_n // PC
    pos_e = dst_n % PC
    p_e = pos_e % P
    g_e = pos_e // P
    col_e = col_of_group[g_e] + slot_in_node
    IDX[core_e, p_e, col_e] = row_of_new(src_n).astype(np.int32)

    # dinv arranged [C, P, G]
    dinv_n = np.zeros(C * PC, dtype=np.float32)
    dinv_n[new_id] = dinv
    dinv_arr = dinv_n.reshape(C, G, P).transpose(0, 2, 1).copy()  # [C, P, G]

    return dict(
        dinv_arr=dinv_arr, IDX=IDX, sgs=sgs, SK=SK,
        old_of_new=old_of_new, new_id=new_id,
    )


# ----------------------------------------------------------------------------
# Device kernel builder
# ----------------------------------------------------------------------------
def _build_program(sgs, SK):
    import concourse.bass as bass
    import concourse.mybir as mybir
    import concourse.tile as tile
    import concourse.bacc as bacc
    from concourse.bass import _add_dep_helper
    from concourse.masks import make_identity

    fp32 = mybir.dt.float32
    bf16 = mybir.dt.bfloat16
    i32 = mybir.dt.int32
    AF = mybir.ActivationFunctionType
    ALU = mybir.AluOpType

    nc = bacc.Bacc("TRN2", target_bir_lowering=False, debug=False,
                   enable_asserts=False, num_devices=C)

    xT = nc.dram_tensor("xT", [IN_DIM, PC], bf16, kind="ExternalInput").ap()
    idx_d = nc.dram_tensor("idx", [P, SK], i32, kind="ExternalInput").ap()
    dinv_d = nc.dram_tensor("dinv", [P, G], fp32, kind="ExternalInput").ap()
    w_d = []
    b_d = []
    dims = [IN_DIM] + DIMS
    for l in range(4):
        w_d.append(nc.dram_tensor(f"w{l+1}", [dims[l], dims[l + 1]], bf16,
                                  kind="ExternalInput").ap())
        b_d.append(nc.dram_tensor(f"b{l+1}", [P, dims[l + 1]], fp32,
                                  kind="ExternalInput").ap())
    out_d = nc.dram_tensor("out", [P, G * 4], fp32, kind="ExternalOutput").ap()

    S0 = SK // len(sgs)
    MSGW = S0 * DIMS[0]          # bf16 msg tile width (also holds xT halves)
    SCRW = (S0 // 2 + 8) * DIMS[0]   # f32 fold scratch width

    with tile.TileContext(nc) as tc:
        with (
            tc.tile_pool(name="const", bufs=1) as cpool,
            tc.tile_pool(name="work", bufs=1) as wpool,
            tc.tile_pool(name="msg", bufs=3) as mpool,
            tc.tile_pool(name="scr", bufs=1) as spool,
            tc.tile_pool(name="mm", bufs=3) as mmpool,
            tc.tile_pool(name="psum", bufs=4, space="PSUM") as pspool,
            tc.tile_pool(name="pst", bufs=4, space="PSUM") as pstpool,
            tc.tile_pool(name="dram", bufs=1, space="DRAM") as dpool,
        ):
            # ---- constants ----
            dinv_sb = cpool.tile([P, G], fp32)
            nc.sync.dma_start(dinv_sb[:], dinv_d[:])
            idx_sb = cpool.tile([P, SK], i32, tag="idx")
            nc.sync.dma_start(idx_sb[:], idx_d[:])
            ident = cpool.tile([P, P], bf16)
            make_identity(nc, ident[:])
            w_sb = []
            b_sb = []
            for l in range(4):
                din, dout = dims[l], dims[l + 1]
                if din > P:
                    wt = cpool.tile([P, (din // P) * dout], bf16, tag=f"w{l}")
                    for cch in range(din // P):
                        nc.sync.dma_start(wt[:, cch * dout:(cch + 1) * dout],
                                          w_d[l][cch * P:(cch + 1) * P, :])
                else:
                    wt = cpool.tile([din, dout], bf16, tag=f"w{l}")
                    nc.sync.dma_start(wt[:], w_d[l][:])
                w_sb.append(wt)
                bt = cpool.tile([P, dout], fp32, tag=f"b{l}")
                nc.sync.dma_start(bt[:], b_d[l][:])
                b_sb.append(bt)

            # persistent accumulator (f32) and table staging (bf16)
            zacc = wpool.tile([P, G * DIMS[0]], fp32, tag="zacc")
            ztab = wpool.tile([P, G * DIMS[0]], bf16, tag="ztab")
            zrelu = wpool.tile([P, G * DIMS[0]], bf16, tag="zrelu")

            tables = []
            for l in range(4):
                tables.append(dpool.tile([C * PG, DIMS[l]], bf16,
                                         name=f"table{l}", tag=f"table{l}",
                                         addr_space="Shared"))
            shards = []
            for l in range(4):
                shards.append(dpool.tile([PG, DIMS[l]], bf16,
                                         name=f"shard{l}", tag=f"shard{l}"))

            def shard_write_and_allgather(l):
                dout = dims[l + 1]
                # row p*G+g  <-  ztab[p, g*dout+d]: dense per-partition runs
                nc.sync.dma_start(
                    shards[l][:].rearrange("(p g) d -> p g d", p=P),
                    ztab[:, :G * dout].rearrange("p (g d) -> p g d", d=dout))
                nc.gpsimd.collective_compute(
                    "AllGather", ALU.bypass,
                    replica_groups=[list(range(C))],
                    ins=[shards[l].opt()], outs=[tables[l].opt()])

            def build_group_table(l, g):
                """One group's table rows for layer l+1 input: transpose zrelu
                slice, matmul with W, dinv-scale into ztab (l >= 1)."""
                din, dout = dims[l], dims[l + 1]
                pst = pstpool.tile([din, P], bf16, tag="trps")
                nc.tensor.transpose(
                    pst[:], zrelu[:, g * din:(g + 1) * din], ident[:])
                zT = mmpool.tile([din, P], bf16, tag="zT")
                nc.vector.tensor_copy(zT[:], pst[:])
                ps = pspool.tile([P, dout], fp32, tag="mmps")
                nc.tensor.matmul(ps[:], lhsT=zT[:], rhs=w_sb[l][:],
                                 start=True, stop=True)
                nc.vector.tensor_scalar_mul(
                    ztab[:, g * dout:(g + 1) * dout], ps[:],
                    dinv_sb[:, g:g + 1])

            def build_table0():
                """Layer-1 table: bulk-load xT into SBUF (2 DMAs), then 2
                accumulating matmuls per group."""
                dout = DIMS[0]
                xa = mpool.tile([P, MSGW], bf16, tag="msg")
                xb = mpool.tile([P, MSGW], bf16, tag="msg")
                nc.sync.dma_start(xa[:, :PC], xT[0:P, :])
                nc.sync.dma_start(xb[:, :PC], xT[P:2 * P, :])
                for t in range(G):
                    ps = pspool.tile([P, dout], fp32, tag="mmps")
                    nc.tensor.matmul(ps[:], lhsT=xa[:, t * P:(t + 1) * P],
                                     rhs=w_sb[0][:, 0:dout],
                                     start=True, stop=False)
                    nc.tensor.matmul(ps[:], lhsT=xb[:, t * P:(t + 1) * P],
                                     rhs=w_sb[0][:, dout:2 * dout],
                                     start=False, stop=True)
                    nc.vector.tensor_scalar_mul(
                        ztab[:, t * dout:(t + 1) * dout], ps[:],
                        dinv_sb[:, t:t + 1])
                shard_write_and_allgather(0)

            scr = spool.tile([P, SCRW], fp32, tag="scr")

            def fold_sg(l, si):
                """Gather + segmented-sum one supergroup into zacc (f32)."""
                dout = dims[l + 1]
                gs, w, K = sgs[si]
                msg = mpool.tile([P, MSGW], bf16, tag="msg")
                gth = nc.gpsimd.indirect_dma_start(
                    out=msg[:, :w * K * dout],
                    out_offset=None,
                    in_=tables[l][:],
                    in_offset=bass.IndirectOffsetOnAxis(
                        ap=idx_sb[:, si * S0:si * S0 + w * K], axis=0),
                )
                m4 = msg[:, :w * K * dout].rearrange(
                    "p (w k d) -> p w k d", w=w, k=K)
                zv = zacc[:, gs * dout:(gs + w) * dout].rearrange(
                    "p (w k d) -> p w k d", w=w, k=1)
                if K == 1:
                    cp = nc.vector.tensor_copy(zv, m4[:, :, 0:1, :])
                    _add_dep_helper(cp.ins, gth.ins, sync=True,
                                    reason="fold waits gather data")
                    return
                if K == 2:
                    tt = nc.vector.tensor_tensor(
                        out=zv, in0=m4[:, :, 0:1, :], in1=m4[:, :, 1:2, :],
                        op=ALU.add)
                    _add_dep_helper(tt.ins, gth.ins, sync=True,
                                    reason="fold waits gather data")
                    return
                # level 1: bf16 pairs -> f32 scratch
                h1 = (K + 1) // 2
                n1 = K - h1
                s4 = scr[:, :w * h1 * dout].rearrange(
                    "p (w k d) -> p w k d", w=w, k=h1)
                tt = nc.vector.tensor_tensor(
                    out=s4[:, :, 0:n1, :], in0=m4[:, :, 0:n1, :],
                    in1=m4[:, :, h1:K, :], op=ALU.add)
                _add_dep_helper(tt.ins, gth.ins, sync=True,
                                reason="fold waits gather data")
                if h1 > n1:
                    cp = nc.vector.tensor_copy(
                        s4[:, :, n1:h1, :], m4[:, :, n1:h1, :])
                    _add_dep_helper(cp.ins, gth.ins, sync=True,
                                    reason="fold waits gather data")
                # f32 tree on scratch
                L = h1
                while L > 1:
                    h = (L + 1) // 2
                    n = L - h
                    tt = nc.vector.tensor_tensor(
                        out=zv if L == 2 else s4[:, :, 0:n, :],
                        in0=s4[:, :, 0:n, :], in1=s4[:, :, h:h + n, :],
                        op=ALU.add)
                    L = h
                if L == 1 and h1 == 1:
                    pass

            def epilogue_sg(l, si):
                """zacc = zacc * dinv + b on one supergroup; relu for l<3."""
                dout = dims[l + 1]
                gs, w, K = sgs[si]
                za3 = zacc[:, gs * dout:(gs + w) * dout].rearrange(
                    "p (w d) -> p w d", d=dout)
                dinv_bc = dinv_sb[:, gs:gs + w].unsqueeze(2).broadcast_to(
                    [P, w, dout])
                nc.vector.tensor_tensor(out=za3, in0=za3, in1=dinv_bc,
                                        op=ALU.mult)
                b_bc = b_sb[l][:].unsqueeze(1).broadcast_to([P, w, dout])
                nc.vector.tensor_tensor(out=za3, in0=za3, in1=b_bc,
                                        op=ALU.add)
                if l < 3:
                    nc.vector.tensor_scalar_max(
                        zrelu[:, gs * dout:(gs + w) * dout],
                        zacc[:, gs * dout:(gs + w) * dout], 0.0)

            # ---------------- pipeline ----------------
            build_table0()
            for l in range(4):
                for si in range(len(sgs)):
                    fold_sg(l, si)
                    epilogue_sg(l, si)
                    if l < 3:
                        gs, w, K = sgs[si]
                        for g in range(gs, gs + w):
                            build_group_table(l + 1, g)
                if l < 3:
                    shard_write_and_allgather(l + 1)

            # ---- log_softmax over d=4 (f32) ----
            za = zacc[:, :G * 4]
            za3 = za.rearrange("p (g d) -> p g d", d=4)
            red = wpool.tile([P, G], fp32, tag="red")
            exps = wpool.tile([P, G * 4], fp32, tag="exps")
            nc.vector.tensor_reduce(out=red[:], in_=za3,
                                    axis=mybir.AxisListType.X, op=ALU.max)
            red_bc = red[:].unsqueeze(2).broadcast_to([P, G, 4])
            nc.vector.tensor_tensor(out=za3, in0=za3, in1=red_bc,
                                    op=ALU.subtract)
            nc.scalar.activation(exps[:], za, AF.Exp)
            nc.vector.tensor_reduce(
                out=red[:], in_=exps[:].rearrange("p (g d) -> p g d", d=4),
                axis=mybir.AxisListType.X, op=ALU.add)
            logs = wpool.tile([P, G], fp32, tag="logs")
            nc.scalar.activation(logs[:], red[:], AF.Ln)
            logs_bc = logs[:].unsqueeze(2).broadcast_to([P, G, 4])
            nc.vector.tensor_tensor(out=za3, in0=za3, in1=logs_bc,
                                    op=ALU.subtract)
            nc.sync.dma_start(out_d[:], za[:])

    nc.compile()
    return nc


# ----------------------------------------------------------------------------
# Entry point
# ----------------------------------------------------------------------------
def kernel(x, edge_index, W1, b1, W2, b2, W3, b3, W4, b4):
    global LAST_RESULT
    from concourse.bass_utils import run_bass_kernel_spmd

    prep = _preprocess(np.asarray(edge_index))
    sgs = tuple(prep["sgs"])
    SK = prep["SK"]

    key = (sgs, SK)
    if key not in _COMPILED:
        _COMPILED[key] = _build_program(list(sgs), SK)
    nc = _COMPILED[key]

    x = np.asarray(x, dtype=np.float32)
    old_of_new = prep["old_of_new"]

    Ws = [np.asarray(w, dtype=np.float32) for w in (W1, W2, W3, W4)]
    bs = [np.asarray(b, dtype=np.float32) for b in (b1, b2, b3, b4)]

    in_maps = []
    for k in range(C):
        ids = old_of_new[k * PC:(k + 1) * PC]
        xk = np.zeros((PC, IN_DIM), dtype=np.float32)
        real = ids >= 0
        xk[real] = x[ids[real]]
        im = {
            "xT": np.ascontiguousarray(xk.T).astype(ml_dtypes.bfloat16),
            "idx": prep["IDX"][k],
            "dinv": prep["dinv_arr"][k],
        }
        for l in range(4):
            im[f"w{l+1}"] = Ws[l].astype(ml_dtypes.bfloat16)
            im[f"b{l+1}"] = np.broadcast_to(bs[l][None, :],
                                            (P, bs[l].shape[0])).copy()
        in_maps.append(im)

    res = run_bass_kernel_spmd(nc, in_maps, core_ids=list(range(C)))
    LAST_RESULT = res

    out = np.zeros((N_NODES, 4), dtype=np.float32)
    for k in range(C):
        ok = np.asarray(res.results[k]["out"], dtype=np.float32)
        # out_d[p, g*4+d] holds node pos = g*128+p
        nodes = ok.reshape(P, G, 4).transpose(1, 0, 2).reshape(PC, 4)
        ids = old_of_new[k * PC:(k + 1) * PC]
        real = ids >= 0
        out[ids[real]] = nodes[real]
    return out


# revision 5
# speedup vs baseline: 1.4541x; 1.4541x over previous
"""4-layer GCN (GroupGCN) on 8 TRN2 NeuronCores.

Strategy:
  - Nodes relabeled by degree rank and striped across the 8 cores (core = rank % 8)
    so every core owns 12500 nodes with an even mix of degrees; edges are assigned
    to the core that owns their dst node.
  - Per layer: each core computes the scaled feature table rows for its own nodes
    (h~ = dinv * (z @ W)) with PE matmuls, all cores AllGather the full table into
    DRAM, then each core aggregates its nodes'
    in-edges with one indirect-DMA gather per supergroup followed by a DVE
    in-place fold-tree segmented sum (fp16 intermediates, f32 final level).
  - All 2-byte data is fp16 (not bf16): the 11-bit mantissa keeps the fold-tree
    rounding error ~1e-3 while retaining the DVE 2x packed throughput mode.
  - Table rows are laid out partition-major (row = c*P*G + p*G + g), so the
    per-layer shard write from SBUF [P, G*dout] is a single dense DMA.
  - PSUM->SBUF moves (zT copy, dinv-scaled ztab) run on the Scalar/ACT engine
    (Copy activation with per-partition scale), keeping DVE free for folds.
  - dinv (D^-1/2) factorizes out of the per-edge norm: table rows are pre-scaled
    by dinv[src], the aggregated sum is post-scaled by dinv[dst].
  - x is bulk-loaded into SBUF once (2 DMAs); the edge index table lives in SBUF
    for the whole kernel; epilogue + next-layer table build run per-supergroup,
    pipelined with the remaining gathers/folds of the current layer.
"""

import numpy as np

N_NODES = 100000
N_EDGES = 3200000
IN_DIM = 256
DIMS = [64, 32, 16, 4]
C = 8           # cores
P = 128         # partitions
G = 98          # node groups of 128 per core
GH = 49         # groups per AllGather chunk (2 chunks)
PC = G * P      # padded nodes per core (12544)
PG = P * G      # table rows per core (= PC)
NPC = N_NODES // C  # real nodes per core (12500)
MSG_SLOT_BUDGET = 224   # max slots (w*K) per supergroup gather

_COMPILED = {}
LAST_RESULT = None


# ----------------------------------------------------------------------------
# Host-side graph preprocessing
# ----------------------------------------------------------------------------
def _row_of_new(n):
    """Table row id of node new-id n: row = c*PG + p*G + g."""
    c = n // PC
    pos = n % PC
    p = pos % P
    g = pos // P
    return c * PG + p * G + g


def _preprocess(edge_index):
    src = edge_index[0].astype(np.int64)
    dst = edge_index[1].astype(np.int64)
    loop = np.arange(N_NODES, dtype=np.int64)
    src = np.concatenate([src, loop])
    dst = np.concatenate([dst, loop])

    deg = np.bincount(dst, minlength=N_NODES).astype(np.float64)
    dinv = np.where(deg > 0, 1.0 / np.sqrt(deg), 0.0).astype(np.float32)

    # degree-rank striping: rank r -> core r % C, slot r // C
    order = np.argsort(deg, kind="stable")          # ascending degree
    core_of_rank = np.arange(N_NODES) % C
    pos_of_rank = np.arange(N_NODES) // C
    new_id = np.empty(N_NODES, dtype=np.int64)
    new_id[order] = core_of_rank * PC + pos_of_rank
    old_of_new = np.full(C * PC, -1, dtype=np.int64)
    old_of_new[new_id] = np.arange(N_NODES)

    src_n = new_id[src]
    dst_n = new_id[dst]

    # sort edges by (dst_new, src_new)
    sort_idx = np.lexsort((src_n, dst_n))
    src_n = src_n[sort_idx]
    dst_n = dst_n[sort_idx]

    # per-node degree in new-id space (length C*PC, dummies have 0)
    deg_n = np.bincount(dst_n, minlength=C * PC)

    # per-(core, group) max degree, then global max across cores -> K_g
    deg_pg = deg_n.reshape(C, G, P)
    K_g = deg_pg.max(axis=(0, 2)).astype(np.int64)  # [G]
    K_g = np.maximum(K_g, 1)

    # pack groups (in order) into supergroups; slots within the IDX array are
    # padded to S0 per supergroup but only the first w*K columns are gathered
    S0 = MSG_SLOT_BUDGET
    sgs = []  # list of (g_start, width, K)
    g0 = 0
    while g0 < G:
        K = int(K_g[g0])
        w = 1
        while (g0 + w < G and
               max(K, int(K_g[g0 + w])) * (w + 1) <= S0 and
               w < 14):
            K = max(K, int(K_g[g0 + w]))
            w += 1
        K = max(K, int(K_g[g0:g0 + w].max()))
        sgs.append((g0, w, K))
        g0 += w

    SK = len(sgs) * S0
    # column offset of each group inside the IDX array
    col_of_group = np.zeros(G, dtype=np.int64)
    for si, (gs, w, K) in enumerate(sgs):
        for i in range(w):
            col_of_group[gs + i] = si * S0 + i * K

    ZERO_ROW = int(_row_of_new(np.int64(NPC)))  # core 0's first dummy node

    # build IDX [C, P, SK] int32 of table row ids
    IDX = np.full((C, P, SK), ZERO_ROW, dtype=np.int32)
    node_starts = np.zeros(C * PC + 1, dtype=np.int64)
    np.cumsum(deg_n, out=node_starts[1:])
    slot_in_node = np.arange(len(dst_n)) - node_starts[dst_n]
    core_e = dst_n // PC
    pos_e = dst_n % PC
    p_e = pos_e % P
    g_e = pos_e // P
    col_e = col_of_group[g_e] + slot_in_node
    IDX[core_e, p_e, col_e] = _row_of_new(src_n).astype(np.int32)

    # dinv arranged [C, P, G]
    dinv_n = np.zeros(C * PC, dtype=np.float32)
    dinv_n[new_id] = dinv
    dinv_arr = dinv_n.reshape(C, G, P).transpose(0, 2, 1).copy()  # [C, P, G]

    return dict(
        dinv_arr=dinv_arr, IDX=IDX, sgs=sgs, SK=SK,
        old_of_new=old_of_new, new_id=new_id,
    )


# ----------------------------------------------------------------------------
# Device kernel builder
# ----------------------------------------------------------------------------
def _build_program(sgs, SK):
    import concourse.bass as bass
    import concourse.mybir as mybir
    import concourse.tile as tile
    import concourse.bacc as bacc
    from concourse.bass import _add_dep_helper
    from concourse.masks import make_identity

    fp32 = mybir.dt.float32
    f16 = mybir.dt.float16
    i32 = mybir.dt.int32
    AF = mybir.ActivationFunctionType
    ALU = mybir.AluOpType

    nc = bacc.Bacc("TRN2", target_bir_lowering=False, debug=False,
                   enable_asserts=False, num_devices=C)

    xT = nc.dram_tensor("xT", [IN_DIM, PC], f16, kind="ExternalInput").ap()
    idx_d = nc.dram_tensor("idx", [P, SK], i32, kind="ExternalInput").ap()
    dinv_d = nc.dram_tensor("dinv", [P, G], fp32, kind="ExternalInput").ap()
    w_d = []
    b_d = []
    dims = [IN_DIM] + DIMS
    for l in range(4):
        w_d.append(nc.dram_tensor(f"w{l+1}", [dims[l], dims[l + 1]], f16,
                                  kind="ExternalInput").ap())
        b_d.append(nc.dram_tensor(f"b{l+1}", [P, dims[l + 1]], fp32,
                                  kind="ExternalInput").ap())
    out_d = nc.dram_tensor("out", [P, G * 4], fp32, kind="ExternalOutput").ap()

    S0 = SK // len(sgs)
    MSGW = S0 * DIMS[0]          # fp16 msg tile width (also holds xT halves)

    with tile.TileContext(nc) as tc:
        with (
            tc.tile_pool(name="const", bufs=1) as cpool,
            tc.tile_pool(name="work", bufs=1) as wpool,
            tc.tile_pool(name="msg", bufs=4) as mpool,
            tc.tile_pool(name="mm", bufs=3) as mmpool,
            tc.tile_pool(name="psum", bufs=4, space="PSUM") as pspool,
            tc.tile_pool(name="pst", bufs=4, space="PSUM") as pstpool,
            tc.tile_pool(name="dram", bufs=1, space="DRAM") as dpool,
        ):
            # ---- constants ----
            dinv_sb = cpool.tile([P, G], fp32)
            nc.sync.dma_start(dinv_sb[:], dinv_d[:])
            idx_sb = cpool.tile([P, SK], i32, tag="idx")
            nc.sync.dma_start(idx_sb[:], idx_d[:])
            ident = cpool.tile([P, P], f16)
            make_identity(nc, ident[:])
            w_sb = []
            b_sb = []
            for l in range(4):
                din, dout = dims[l], dims[l + 1]
                if din > P:
                    wt = cpool.tile([P, (din // P) * dout], f16, tag=f"w{l}")
                    for cch in range(din // P):
                        nc.sync.dma_start(wt[:, cch * dout:(cch + 1) * dout],
                                          w_d[l][cch * P:(cch + 1) * P, :])
                else:
                    wt = cpool.tile([din, dout], f16, tag=f"w{l}")
                    nc.sync.dma_start(wt[:], w_d[l][:])
                w_sb.append(wt)
                bt = cpool.tile([P, dout], fp32, tag=f"b{l}")
                nc.sync.dma_start(bt[:], b_d[l][:])
                b_sb.append(bt)

            # persistent accumulator (f32) and table staging (fp16)
            zacc = wpool.tile([P, G * DIMS[0]], fp32, tag="zacc")
            ztab = wpool.tile([P, G * DIMS[0]], f16, tag="ztab")
            zrelu = wpool.tile([P, G * DIMS[0]], f16, tag="zrelu")

            tables = []
            for l in range(4):
                tables.append(dpool.tile([C * PG, DIMS[l]], f16,
                                         name=f"table{l}", tag=f"table{l}",
                                         addr_space="Shared"))
            shards = []
            for l in range(4):
                shards.append(dpool.tile([PG, DIMS[l]], f16,
                                         name=f"shard{l}", tag=f"shard{l}"))

            def shard_ag(l):
                """Write ztab to the shard (dense DMA) and AllGather it."""
                dout = dims[l + 1]
                nc.sync.dma_start(
                    shards[l][:].rearrange("(p g) d -> p g d", p=P),
                    ztab[:, :G * dout].rearrange("p (g d) -> p g d", d=dout))
                nc.gpsimd.collective_compute(
                    "AllGather", ALU.bypass,
                    replica_groups=[list(range(C))],
                    ins=[shards[l].opt()], outs=[tables[l].opt()])

            def build_group_table(l, g):
                """One group's table rows for layer l+1 input: PE-transpose the
                zrelu slice, matmul with W, dinv-scale into ztab via ACT."""
                din, dout = dims[l], dims[l + 1]
                pst = pstpool.tile([din, P], f16, tag="trps")
                nc.tensor.transpose(
                    pst[:], zrelu[:, g * din:(g + 1) * din], ident[:])
                zT = mmpool.tile([din, P], f16, tag="zT")
                nc.scalar.activation(zT[:], pst[:], AF.Copy)
                ps = pspool.tile([P, dout], fp32, tag="mmps")
                nc.tensor.matmul(ps[:], lhsT=zT[:], rhs=w_sb[l][:],
                                 start=True, stop=True)
                nc.scalar.activation(
                    ztab[:, g * dout:(g + 1) * dout], ps[:], AF.Copy,
                    scale=dinv_sb[:, g:g + 1])

            def build_table0():
                """Layer-1 table: bulk-load xT into SBUF (2 DMAs), then 2
                accumulating matmuls per group; AllGather in 2 chunks."""
                dout = DIMS[0]
                xa = mpool.tile([P, MSGW], f16, tag="msg")
                xb = mpool.tile([P, MSGW], f16, tag="msg")
                nc.sync.dma_start(xa[:, :PC], xT[0:P, :])
                nc.sync.dma_start(xb[:, :PC], xT[P:2 * P, :])
                for t in range(G):
                    ps = pspool.tile([P, dout], fp32, tag="mmps")
                    nc.tensor.matmul(ps[:], lhsT=xa[:, t * P:(t + 1) * P],
                                     rhs=w_sb[0][:, 0:dout],
                                     start=True, stop=False)
                    nc.tensor.matmul(ps[:], lhsT=xb[:, t * P:(t + 1) * P],
                                     rhs=w_sb[0][:, dout:2 * dout],
                                     start=False, stop=True)
                    nc.scalar.activation(
                        ztab[:, t * dout:(t + 1) * dout], ps[:], AF.Copy,
                        scale=dinv_sb[:, t:t + 1])
                shard_ag(0)

            def fold_sg(l, si):
                """Gather + in-place fold-tree segmented sum into zacc."""
                dout = dims[l + 1]
                gs, w, K = sgs[si]
                msg = mpool.tile([P, MSGW], f16, tag="msg")
                gth = nc.gpsimd.indirect_dma_start(
                    out=msg[:, :w * K * dout],
                    out_offset=None,
                    in_=tables[l][:],
                    in_offset=bass.IndirectOffsetOnAxis(
                        ap=idx_sb[:, si * S0:si * S0 + w * K], axis=0),
                )
                m4 = msg[:, :w * K * dout].rearrange(
                    "p (w k d) -> p w k d", w=w, k=K)
                zv = zacc[:, gs * dout:(gs + w) * dout].rearrange(
                    "p (w k d) -> p w k d", w=w, k=1)
                if K == 1:
                    cp = nc.vector.tensor_copy(zv, m4[:, :, 0:1, :])
                    _add_dep_helper(cp.ins, gth.ins, sync=True,
                                    reason="fold waits gather data")
                    return
                first = True
                L = K
                while L > 1:
                    h = (L + 1) // 2
                    n = L - h
                    tt = nc.vector.tensor_tensor(
                        out=zv if L == 2 else m4[:, :, 0:n, :],
                        in0=m4[:, :, 0:n, :], in1=m4[:, :, h:h + n, :],
                        op=ALU.add)
                    if first:
                        _add_dep_helper(tt.ins, gth.ins, sync=True,
                                        reason="fold waits gather data")
                        first = False
                    L = h

            def epilogue_sg(l, si):
                """zacc = zacc * dinv + b on one supergroup; relu for l<3."""
                dout = dims[l + 1]
                gs, w, K = sgs[si]
                za3 = zacc[:, gs * dout:(gs + w) * dout].rearrange(
                    "p (w d) -> p w d", d=dout)
                dinv_bc = dinv_sb[:, gs:gs + w].unsqueeze(2).broadcast_to(
                    [P, w, dout])
                nc.vector.tensor_tensor(out=za3, in0=za3, in1=dinv_bc,
                                        op=ALU.mult)
                b_bc = b_sb[l][:].unsqueeze(1).broadcast_to([P, w, dout])
                nc.vector.tensor_tensor(out=za3, in0=za3, in1=b_bc,
                                        op=ALU.add)
                if l < 3:
                    nc.vector.tensor_scalar_max(
                        zrelu[:, gs * dout:(gs + w) * dout],
                        zacc[:, gs * dout:(gs + w) * dout], 0.0)

            # ---------------- pipeline ----------------
            build_table0()
            for l in range(4):
                for si in range(len(sgs)):
                    fold_sg(l, si)
                    epilogue_sg(l, si)
                    if l < 3:
                        gs, w, K = sgs[si]
                        for g in range(gs, gs + w):
                            build_group_table(l + 1, g)
                if l < 3:
                    shard_ag(l + 1)

            # ---- log_softmax over d=4 (f32) ----
            za = zacc[:, :G * 4]
            za3 = za.rearrange("p (g d) -> p g d", d=4)
            red = wpool.tile([P, G], fp32, tag="red")
            exps = wpool.tile([P, G * 4], fp32, tag="exps")
            nc.vector.tensor_reduce(out=red[:], in_=za3,
                                    axis=mybir.AxisListType.X, op=ALU.max)
            red_bc = red[:].unsqueeze(2).broadcast_to([P, G, 4])
            nc.vector.tensor_tensor(out=za3, in0=za3, in1=red_bc,
                                    op=ALU.subtract)
            nc.scalar.activation(exps[:], za, AF.Exp)
            nc.vector.tensor_reduce(
                out=red[:], in_=exps[:].rearrange("p (g d) -> p g d", d=4),
                axis=mybir.AxisListType.X, op=ALU.add)
            logs = wpool.tile([P, G], fp32, tag="logs")
            nc.scalar.activation(logs[:], red[:], AF.Ln)
            logs_bc = logs[:].unsqueeze(2).broadcast_to([P, G, 4])
            nc.vector.tensor_tensor(out=za3, in0=za3, in1=logs_bc,
                                    op=ALU.subtract)
            nc.sync.dma_start(out_d[:], za[:])

    nc.compile()
    return nc


# ----------------------------------------------------------------------------
# Entry point
# ----------------------------------------------------------------------------
def kernel(x, edge_index, W1, b1, W2, b2, W3, b3, W4, b4):
    global LAST_RESULT
    from concourse.bass_utils import run_bass_kernel_spmd

    prep = _preprocess(np.asarray(edge_index))
    sgs = tuple(prep["sgs"])
    SK = prep["SK"]

    key = (sgs, SK)
    if key not in _COMPILED:
        _COMPILED[key] = _build_program(list(sgs), SK)
    nc = _COMPILED[key]

    x = np.asarray(x, dtype=np.float32)
    old_of_new = prep["old_of_new"]

    Ws = [np.asarray(w, dtype=np.float32) for w in (W1, W2, W3, W4)]
    bs = [np.asarray(b, dtype=np.float32) for b in (b1, b2, b3, b4)]

    in_maps = []
    for k in range(C):
        ids = old_of_new[k * PC:(k + 1) * PC]
        xk = np.zeros((PC, IN_DIM), dtype=np.float32)
        real = ids >= 0
        xk[real] = x[ids[real]]
        im = {
            "xT": np.ascontiguousarray(xk.T).astype(np.float16),
            "idx": prep["IDX"][k],
            "dinv": prep["dinv_arr"][k],
        }
        for l in range(4):
            im[f"w{l+1}"] = Ws[l].astype(np.float16)
            im[f"b{l+1}"] = np.broadcast_to(bs[l][None, :],
                                            (P, bs[l].shape[0])).copy()
        in_maps.append(im)

    res = run_bass_kernel_spmd(nc, in_maps, core_ids=list(range(C)))
    LAST_RESULT = res

    out = np.zeros((N_NODES, 4), dtype=np.float32)
    for k in range(C):
        ok = np.asarray(res.results[k]["out"], dtype=np.float32)
        # out_d[p, g*4+d] holds node pos = g*128+p
        nodes = ok.reshape(P, G, 4).transpose(1, 0, 2).reshape(PC, 4)
        ids = old_of_new[k * PC:(k + 1) * PC]
        real = ids >= 0
        out[ids[real]] = nodes[real]
    return out


# revision 11
# speedup vs baseline: 1.4888x; 1.0238x over previous
"""4-layer GCN (GroupGCN) on 8 TRN2 NeuronCores.

Strategy:
  - Nodes relabeled by degree rank and striped across the 8 cores (core = rank % 8)
    so every core owns 12500 nodes with an even mix of degrees; edges are assigned
    to the core that owns their dst node.
  - Per layer: each core computes the scaled feature table rows for its own nodes
    (h~ = dinv * (z @ W)) with PE matmuls, all cores AllGather the full table into
    DRAM, then each core aggregates its nodes'
    in-edges with one indirect-DMA gather per supergroup followed by a DVE
    in-place fold-tree segmented sum (fp16 intermediates, f32 final level).
  - All 2-byte data is fp16 (not bf16): the 11-bit mantissa keeps the fold-tree
    rounding error ~1e-3 while retaining the DVE 2x packed throughput mode.
  - Table rows are laid out partition-major (row = c*P*G + p*G + g), so the
    per-layer shard write from SBUF [P, G*dout] is a single dense DMA.
  - PSUM->SBUF moves (zT copy, dinv-scaled ztab) run on the Scalar/ACT engine
    (Copy activation with per-partition scale), keeping DVE free for folds.
  - dinv (D^-1/2) factorizes out of the per-edge norm: table rows are pre-scaled
    by dinv[src], the aggregated sum is post-scaled by dinv[dst].
  - x is bulk-loaded into SBUF once (2 DMAs); the edge index table lives in SBUF
    for the whole kernel; epilogue + next-layer table build run per-supergroup,
    pipelined with the remaining gathers/folds of the current layer.
"""

import numpy as np

N_NODES = 100000
N_EDGES = 3200000
IN_DIM = 256
DIMS = [64, 32, 16, 4]
C = 8           # cores
P = 128         # partitions
G = 98          # node groups of 128 per core
GH = 49         # groups per AllGather chunk (2 chunks)
PC = G * P      # padded nodes per core (12544)
PG = P * G      # table rows per core (= PC)
NPC = N_NODES // C  # real nodes per core (12500)
MSG_SLOT_BUDGET = 224   # max slots (w*K) per supergroup gather

_COMPILED = {}
LAST_RESULT = None


# ----------------------------------------------------------------------------
# Host-side graph preprocessing
# ----------------------------------------------------------------------------
def _row_of_new(n):
    """Table row id of node new-id n: row = c*PG + p*G + g."""
    c = n // PC
    pos = n % PC
    p = pos % P
    g = pos // P
    return c * PG + p * G + g


def _preprocess(edge_index):
    src = edge_index[0].astype(np.int64)
    dst = edge_index[1].astype(np.int64)
    loop = np.arange(N_NODES, dtype=np.int64)
    src = np.concatenate([src, loop])
    dst = np.concatenate([dst, loop])

    deg = np.bincount(dst, minlength=N_NODES).astype(np.float64)
    dinv = np.where(deg > 0, 1.0 / np.sqrt(deg), 0.0).astype(np.float32)

    # degree-rank striping: rank r -> core r % C, slot r // C
    order = np.argsort(deg, kind="stable")          # ascending degree
    core_of_rank = np.arange(N_NODES) % C
    pos_of_rank = np.arange(N_NODES) // C
    new_id = np.empty(N_NODES, dtype=np.int64)
    new_id[order] = core_of_rank * PC + pos_of_rank
    old_of_new = np.full(C * PC, -1, dtype=np.int64)
    old_of_new[new_id] = np.arange(N_NODES)

    src_n = new_id[src]
    dst_n = new_id[dst]

    # sort edges by (dst_new, src_new)
    sort_idx = np.lexsort((src_n, dst_n))
    src_n = src_n[sort_idx]
    dst_n = dst_n[sort_idx]

    # per-node degree in new-id space (length C*PC, dummies have 0)
    deg_n = np.bincount(dst_n, minlength=C * PC)

    # per-(core, group) max degree, then global max across cores -> K_g
    deg_pg = deg_n.reshape(C, G, P)
    K_g = deg_pg.max(axis=(0, 2)).astype(np.int64)  # [G]
    K_g = np.maximum(K_g, 1)

    # pack groups (in order) into supergroups; slots within the IDX array are
    # padded to S0 per supergroup but only the first w*K columns are gathered
    S0 = MSG_SLOT_BUDGET
    sgs = []  # list of (g_start, width, K)
    g0 = 0
    while g0 < G:
        K = int(K_g[g0])
        w = 1
        while (g0 + w < G and
               max(K, int(K_g[g0 + w])) * (w + 1) <= S0 and
               w < 14):
            K = max(K, int(K_g[g0 + w]))
            w += 1
        K = max(K, int(K_g[g0:g0 + w].max()))
        sgs.append((g0, w, K))
        g0 += w

    SK = len(sgs) * S0
    # k-major slot layout inside each supergroup: slot (k, wi) of group
    # gs+wi sits at column si*S0 + k*w + wi, so every fold-tree level is a
    # contiguous 2D range on the device
    sg_col0 = np.zeros(G, dtype=np.int64)
    sg_w = np.zeros(G, dtype=np.int64)
    sg_gs = np.zeros(G, dtype=np.int64)
    for si, (gs, w, K) in enumerate(sgs):
        sg_col0[gs:gs + w] = si * S0
        sg_w[gs:gs + w] = w
        sg_gs[gs:gs + w] = gs

    ZERO_ROW = int(_row_of_new(np.int64(NPC)))  # core 0's first dummy node

    # build IDX [C, P, SK] int32 of table row ids
    IDX = np.full((C, P, SK), ZERO_ROW, dtype=np.int32)
    node_starts = np.zeros(C * PC + 1, dtype=np.int64)
    np.cumsum(deg_n, out=node_starts[1:])
    slot_in_node = np.arange(len(dst_n)) - node_starts[dst_n]
    core_e = dst_n // PC
    pos_e = dst_n % PC
    p_e = pos_e % P
    g_e = pos_e // P
    col_e = sg_col0[g_e] + slot_in_node * sg_w[g_e] + (g_e - sg_gs[g_e])
    IDX[core_e, p_e, col_e] = _row_of_new(src_n).astype(np.int32)

    # dinv arranged [C, P, G]
    dinv_n = np.zeros(C * PC, dtype=np.float32)
    dinv_n[new_id] = dinv
    dinv_arr = dinv_n.reshape(C, G, P).transpose(0, 2, 1).copy()  # [C, P, G]

    return dict(
        dinv_arr=dinv_arr, IDX=IDX, sgs=sgs, SK=SK,
        old_of_new=old_of_new, new_id=new_id,
    )


# ----------------------------------------------------------------------------
# Device kernel builder
# ----------------------------------------------------------------------------
def _build_program(sgs, SK):
    import concourse.bass as bass
    import concourse.mybir as mybir
    import concourse.tile as tile
    import concourse.bacc as bacc
    from concourse.bass import _add_dep_helper
    from concourse.masks import make_identity

    fp32 = mybir.dt.float32
    f16 = mybir.dt.float16
    i32 = mybir.dt.int32
    AF = mybir.ActivationFunctionType
    ALU = mybir.AluOpType

    nc = bacc.Bacc("TRN2", target_bir_lowering=False, debug=False,
                   enable_asserts=False, num_devices=C)

    xT = nc.dram_tensor("xT", [IN_DIM, PC], f16, kind="ExternalInput").ap()
    idx_d = nc.dram_tensor("idx", [P, SK], i32, kind="ExternalInput").ap()
    dinv_d = nc.dram_tensor("dinv", [P, G], fp32, kind="ExternalInput").ap()
    w_d = []
    b_d = []
    dims = [IN_DIM] + DIMS
    for l in range(4):
        w_d.append(nc.dram_tensor(f"w{l+1}", [dims[l], dims[l + 1]], f16,
                                  kind="ExternalInput").ap())
        b_d.append(nc.dram_tensor(f"b{l+1}", [P, dims[l + 1]], fp32,
                                  kind="ExternalInput").ap())
    out_d = nc.dram_tensor("out", [P, G * 4], fp32, kind="ExternalOutput").ap()

    S0 = SK // len(sgs)
    MSGW = S0 * DIMS[0]          # fp16 msg tile width (also holds xT halves)

    with tile.TileContext(nc) as tc:
        with (
            tc.tile_pool(name="const", bufs=1) as cpool,
            tc.tile_pool(name="work", bufs=1) as wpool,
            tc.tile_pool(name="msg", bufs=4) as mpool,
            tc.tile_pool(name="mm", bufs=3) as mmpool,
            tc.tile_pool(name="psum", bufs=4, space="PSUM") as pspool,
            tc.tile_pool(name="pst", bufs=4, space="PSUM") as pstpool,
            tc.tile_pool(name="dram", bufs=1, space="DRAM") as dpool,
        ):
            # ---- constants ----
            dinv_sb = cpool.tile([P, G], fp32)
            nc.sync.dma_start(dinv_sb[:], dinv_d[:])
            idx_sb = cpool.tile([P, SK], i32, tag="idx")
            nc.sync.dma_start(idx_sb[:], idx_d[:])
            ident = cpool.tile([P, P], f16)
            make_identity(nc, ident[:])
            w_sb = []
            b_sb = []
            for l in range(4):
                din, dout = dims[l], dims[l + 1]
                if din > P:
                    wt = cpool.tile([P, (din // P) * dout], f16, tag=f"w{l}")
                    for cch in range(din // P):
                        nc.sync.dma_start(wt[:, cch * dout:(cch + 1) * dout],
                                          w_d[l][cch * P:(cch + 1) * P, :])
                else:
                    wt = cpool.tile([din, dout], f16, tag=f"w{l}")
                    nc.sync.dma_start(wt[:], w_d[l][:])
                w_sb.append(wt)
                bt = cpool.tile([P, dout], fp32, tag=f"b{l}")
                nc.sync.dma_start(bt[:], b_d[l][:])
                b_sb.append(bt)

            # persistent accumulator (f32) and table staging (fp16)
            zacc = wpool.tile([P, G * DIMS[0]], fp32, tag="zacc")
            ztab = wpool.tile([P, G * DIMS[0]], f16, tag="ztab")
            zrelu = wpool.tile([P, G * DIMS[0]], f16, tag="zrelu")

            tables = []
            for l in range(4):
                tables.append(dpool.tile([C * PG, DIMS[l]], f16,
                                         name=f"table{l}", tag=f"table{l}",
                                         addr_space="Shared"))
            shards = []
            for l in range(4):
                shards.append(dpool.tile([PG, DIMS[l]], f16,
                                         name=f"shard{l}", tag=f"shard{l}"))

            def shard_ag(l):
                """Write ztab to the shard (dense DMA) and AllGather it."""
                dout = dims[l + 1]
                nc.sync.dma_start(
                    shards[l][:].rearrange("(p g) d -> p g d", p=P),
                    ztab[:, :G * dout].rearrange("p (g d) -> p g d", d=dout))
                nc.gpsimd.collective_compute(
                    "AllGather", ALU.bypass,
                    replica_groups=[list(range(C))],
                    ins=[shards[l].opt()], outs=[tables[l].opt()])

            def build_group_table(l, g):
                """One group's table rows for layer l+1 input: PE-transpose the
                zrelu slice, matmul with W, dinv-scale into ztab via ACT."""
                din, dout = dims[l], dims[l + 1]
                pst = pstpool.tile([din, P], f16, tag="trps")
                nc.tensor.transpose(
                    pst[:], zrelu[:, g * din:(g + 1) * din], ident[:])
                zT = mmpool.tile([din, P], f16, tag="zT")
                nc.scalar.activation(zT[:], pst[:], AF.Copy)
                ps = pspool.tile([P, dout], fp32, tag="mmps")
                nc.tensor.matmul(ps[:], lhsT=zT[:], rhs=w_sb[l][:],
                                 start=True, stop=True)
                nc.scalar.activation(
                    ztab[:, g * dout:(g + 1) * dout], ps[:], AF.Copy,
                    scale=dinv_sb[:, g:g + 1])

            def build_table0():
                """Layer-1 table: bulk-load xT into SBUF (2 DMAs), then 2
                accumulating matmuls per group; AllGather in 2 chunks."""
                dout = DIMS[0]
                xa = mpool.tile([P, MSGW], f16, tag="msg")
                xb = mpool.tile([P, MSGW], f16, tag="msg")
                half = PC // 2
                nc.sync.dma_start(xa[:, :half], xT[0:P, 0:half])
                nc.sync.dma_start(xb[:, :half], xT[P:2 * P, 0:half])
                nc.sync.dma_start(xa[:, half:PC], xT[0:P, half:PC])
                nc.sync.dma_start(xb[:, half:PC], xT[P:2 * P, half:PC])
                for t in range(G):
                    ps = pspool.tile([P, dout], fp32, tag="mmps")
                    nc.tensor.matmul(ps[:], lhsT=xa[:, t * P:(t + 1) * P],
                                     rhs=w_sb[0][:, 0:dout],
                                     start=True, stop=False)
                    nc.tensor.matmul(ps[:], lhsT=xb[:, t * P:(t + 1) * P],
                                     rhs=w_sb[0][:, dout:2 * dout],
                                     start=False, stop=True)
                    nc.scalar.activation(
                        ztab[:, t * dout:(t + 1) * dout], ps[:], AF.Copy,
                        scale=dinv_sb[:, t:t + 1])
                shard_ag(0)

            def fold_sg(l, si):
                """Gather + in-place fold-tree segmented sum into zacc.

                Slots are k-major (slot k of the supergroup's w groups are
                adjacent), so every tree level adds two flat contiguous 2D
                ranges — eligible for the DVE packed fast path."""
                dout = dims[l + 1]
                gs, w, K = sgs[si]
                wd = w * dout
                msg = mpool.tile([P, MSGW], f16, tag="msg")
                gth = nc.gpsimd.indirect_dma_start(
                    out=msg[:, :K * wd],
                    out_offset=None,
                    in_=tables[l][:],
                    in_offset=bass.IndirectOffsetOnAxis(
                        ap=idx_sb[:, si * S0:si * S0 + K * w], axis=0),
                )
                zv = zacc[:, gs * dout:(gs + w) * dout]
                if K == 1:
                    cp = nc.vector.tensor_copy(zv, msg[:, :wd])
                    _add_dep_helper(cp.ins, gth.ins, sync=True,
                                    reason="fold waits gather data")
                    return
                first = True
                L = K
                while L > 1:
                    h = (L + 1) // 2
                    n = L - h
                    tt = nc.vector.tensor_tensor(
                        out=zv if L == 2 else msg[:, :n * wd],
                        in0=msg[:, :n * wd],
                        in1=msg[:, h * wd:(h + n) * wd],
                        op=ALU.add)
                    if first:
                        _add_dep_helper(tt.ins, gth.ins, sync=True,
                                        reason="fold waits gather data")
                        first = False
                    L = h

            def epilogue(l, g0, ng, relu):
                """zacc = zacc * dinv + b on groups [g0, g0+ng); relu opt."""
                dout = dims[l + 1]
                za3 = zacc[:, g0 * dout:(g0 + ng) * dout].rearrange(
                    "p (w d) -> p w d", d=dout)
                dinv_bc = dinv_sb[:, g0:g0 + ng].unsqueeze(2).broadcast_to(
                    [P, ng, dout])
                nc.vector.tensor_tensor(out=za3, in0=za3, in1=dinv_bc,
                                        op=ALU.mult)
                b_bc = b_sb[l][:].unsqueeze(1).broadcast_to([P, ng, dout])
                nc.vector.tensor_tensor(out=za3, in0=za3, in1=b_bc,
                                        op=ALU.add)
                if relu:
                    nc.vector.tensor_scalar_max(
                        zrelu[:, g0 * dout:(g0 + ng) * dout],
                        zacc[:, g0 * dout:(g0 + ng) * dout], 0.0)

            # ---------------- pipeline ----------------
            # tiny warmup collective to absorb the first-CC trigger latency
            warm_in = dpool.tile([16, 16], f16, name="warm_in", tag="warm_in")
            warm_out = dpool.tile([C * 16, 16], f16, name="warm_out",
                                  tag="warm_out", addr_space="Shared")
            nc.gpsimd.collective_compute(
                "AllGather", ALU.bypass, replica_groups=[list(range(C))],
                ins=[warm_in.opt()], outs=[warm_out.opt()])
            build_table0()
            for l in range(4):
                for si in range(len(sgs)):
                    fold_sg(l, si)
                    if l < 3:
                        gs, w, K = sgs[si]
                        epilogue(l, gs, w, relu=True)
                        for g in range(gs, gs + w):
                            build_group_table(l + 1, g)
                if l < 3:
                    shard_ag(l + 1)
                else:
                    epilogue(l, 0, G, relu=False)

            # ---- log_softmax over d=4 (f32) ----
            za = zacc[:, :G * 4]
            za3 = za.rearrange("p (g d) -> p g d", d=4)
            red = wpool.tile([P, G], fp32, tag="red")
            exps = wpool.tile([P, G * 4], fp32, tag="exps")
            nc.vector.tensor_reduce(out=red[:], in_=za3,
                                    axis=mybir.AxisListType.X, op=ALU.max)
            red_bc = red[:].unsqueeze(2).broadcast_to([P, G, 4])
            nc.vector.tensor_tensor(out=za3, in0=za3, in1=red_bc,
                                    op=ALU.subtract)
            nc.scalar.activation(exps[:], za, AF.Exp)
            nc.vector.tensor_reduce(
                out=red[:], in_=exps[:].rearrange("p (g d) -> p g d", d=4),
                axis=mybir.AxisListType.X, op=ALU.add)
            logs = wpool.tile([P, G], fp32, tag="logs")
            nc.scalar.activation(logs[:], red[:], AF.Ln)
            logs_bc = logs[:].unsqueeze(2).broadcast_to([P, G, 4])
            nc.vector.tensor_tensor(out=za3, in0=za3, in1=logs_bc,
                                    op=ALU.subtract)
            nc.sync.dma_start(out_d[:], za[:])

    nc.compile()
    return nc


# ----------------------------------------------------------------------------
# Entry point
# ----------------------------------------------------------------------------
def kernel(x, edge_index, W1, b1, W2, b2, W3, b3, W4, b4):
    global LAST_RESULT
    from concourse.bass_utils import run_bass_kernel_spmd

    prep = _preprocess(np.asarray(edge_index))
    sgs = tuple(prep["sgs"])
    SK = prep["SK"]

    key = (sgs, SK)
    if key not in _COMPILED:
        _COMPILED[key] = _build_program(list(sgs), SK)
    nc = _COMPILED[key]

    x = np.asarray(x, dtype=np.float32)
    old_of_new = prep["old_of_new"]

    Ws = [np.asarray(w, dtype=np.float32) for w in (W1, W2, W3, W4)]
    bs = [np.asarray(b, dtype=np.float32) for b in (b1, b2, b3, b4)]

    in_maps = []
    for k in range(C):
        ids = old_of_new[k * PC:(k + 1) * PC]
        xk = np.zeros((PC, IN_DIM), dtype=np.float32)
        real = ids >= 0
        xk[real] = x[ids[real]]
        im = {
            "xT": np.ascontiguousarray(xk.T).astype(np.float16),
            "idx": prep["IDX"][k],
            "dinv": prep["dinv_arr"][k],
        }
        for l in range(4):
            im[f"w{l+1}"] = Ws[l].astype(np.float16)
            im[f"b{l+1}"] = np.broadcast_to(bs[l][None, :],
                                            (P, bs[l].shape[0])).copy()
        in_maps.append(im)

    res = run_bass_kernel_spmd(nc, in_maps, core_ids=list(range(C)))
    LAST_RESULT = res

    out = np.zeros((N_NODES, 4), dtype=np.float32)
    for k in range(C):
        ok = np.asarray(res.results[k]["out"], dtype=np.float32)
        # out_d[p, g*4+d] holds node pos = g*128+p
        nodes = ok.reshape(P, G, 4).transpose(1, 0, 2).reshape(PC, 4)
        ids = old_of_new[k * PC:(k + 1) * PC]
        real = ids >= 0
        out[ids[real]] = nodes[real]
    return out
